# revision 22
# baseline (speedup 1.0000x reference)
"""Trainium2 Bass kernel for nn_CachedCompressedLinear.

out[16, 11008] = x[16, 4096] @ ((w_q - 128) * scale).T + bias

Sharding: column-parallel over 8 NeuronCores; each core owns a 1376-wide
slice of out_features (8 * 1376 = 11008).

The int32 weight codes are packed to uint8 on the host (values are 0..255,
so the upper 3 bytes in HBM are zeros), cutting weight DMA 4x to
5.64 MB/core.  On-device decode uint8 -> bf16 computes (c - 128) * s
directly (scale and shift fused into the decode) and is split between DVE
(cols 0:864 of each k-tile, 2x perf mode) and ACT (cols 864:1376, 1x);
group sizes are 1/2/4 k-tiles (odd k-tile counts break the DVE 2x mode)
so supply stays just ahead of the PE.  x is replicated and pre-transposed
to plain bf16 (the bf16 rounding of x and of the scaled weights together
cost ~2e-3 relative error against a 2e-2 budget), so each k-tile is one
16-column stationary load and the epilogue is a single PSUM->SBUF copy
per chunk.  Bias is host-split into bf16 hi/lo rows and folded into the
PSUM accumulation by one K=2 matmul per chunk against a two-row one-hot
block at k=24.  Dummy matmuls on a memset tile warm the PE's HAM clock
gate toward 2.4 GHz before the first real matmul; the k=0 weights ride
the scalar HWDGE ring (short receipt) while the rest stream on the
gpsimd SWDGE ring, and x/scale/bias ride the sync ring.
"""

import sys

if "/opt/trn_rl_repo" not in sys.path:
    sys.path.insert(0, "/opt/trn_rl_repo")

import numpy as np
import ml_dtypes

IN_F = 4096
OUT_F = 11008
BATCH = 16
N_CORES = 8
O_PER = 1376  # out_features per core
K_TILES = IN_F // 128  # 32
M = 16  # stationary columns: x in bf16
# (offset, width, engine): DVE decodes [0, 864), ACT decodes [864, 1376)
CHUNKS = [(0, 512, "dve"), (512, 352, "dve"), (864, 512, "act")]
DVE_W = 864
ACT_W = 512
# k-tile counts per SWDGE transfer (k=0 goes separately via scalar HWDGE):
# small at the ends to fill/drain the pipeline, 4-tile in the middle
GROUPS = [1, 2, 2, 4, 4, 4, 4, 4, 2, 2, 1, 1]
BIAS_K = 24  # k-tile at which the bias matmuls are folded in
N_WARM = 15  # dummy matmuls to warm the PE clock gate
WARM_N = 256  # moving width of each warm matmul

_BUILT = None


def _build():
    """Build the (SPMD, per-core) Bass program once."""
    import concourse.bass as bass
    import concourse.tile as tile
    from concourse import bacc, mybir

    dt = mybir.dt
    alu = mybir.AluOpType
    nc = bacc.Bacc("TRN2", target_bir_lowering=False, debug=False)
    # no instruction in this kernel uses the engine library consts
    # (iota/reduce/TT tables) — skip the ~1.1us preamble TENSOR_LOAD
    nc.insert_library_loads = lambda: None

    wt8 = nc.dram_tensor("wt8", [128, K_TILES * O_PER], dt.uint8,
                         kind="ExternalInput")
    xt2 = nc.dram_tensor(
        "xt2", [128, (K_TILES + 1) * M], dt.bfloat16, kind="ExternalInput"
    )
    bias_hl = nc.dram_tensor(
        "bias_hl", [2, O_PER], dt.bfloat16, kind="ExternalInput"
    )
    # col 0: s (replicated), col 1: -128*s
    sc2 = nc.dram_tensor("sc2", [128, 2], dt.float32, kind="ExternalInput")
    out = nc.dram_tensor("out", [BATCH, O_PER], dt.float32, kind="ExternalOutput")

    BIASBLK = K_TILES  # extra stationary block holding the bias one-hot
    with tile.TileContext(nc) as tc:
        with (
            tc.tile_pool(name="consts", bufs=1) as consts,
            tc.tile_pool(name="w8", bufs=1) as w8p,
            tc.tile_pool(name="wbA", bufs=4) as wbAp,
            tc.tile_pool(name="wbB", bufs=4) as wbBp,
            tc.tile_pool(name="psum", bufs=1, space=bass.MemorySpace.PSUM) as psump,
            tc.tile_pool(name="outp", bufs=1) as outp,
        ):
            # sync ring: scale (gates every decode), then the k=0 weights
            # (HWDGE receipt is ~1.5us shorter than SWDGE, so the first
            # decode starts early), then bias (needed at k=BIAS_K).
            # scalar ring: x (gates the first matmul).  gpsimd ring:
            # k=1..31 weights.
            sc_sb = consts.tile([128, 2], dt.float32)
            nc.sync.dma_start(sc_sb[:], sc2[:])
            x_sb = consts.tile([128, (K_TILES + 1) * M], dt.bfloat16)
            nc.scalar.dma_start(x_sb[:], xt2[:])

            GMAX = max(GROUPS)
            w_tiles = []
            wt_0 = w8p.tile([128, 1, O_PER], dt.uint8, tag="w8_k0")
            nc.sync.dma_start(wt_0[:, 0:1, :], wt8[:, 0:O_PER])
            w_tiles.append((0, 1, wt_0))
            bias_sb = consts.tile([2, O_PER], dt.bfloat16)
            nc.sync.dma_start(bias_sb[:], bias_hl[:])
            k0 = 1
            for gi, G in enumerate(GROUPS):
                wt_t = w8p.tile([128, G, O_PER], dt.uint8, tag=f"w8_{gi}")
                nc.gpsimd.dma_start(
                    wt_t[:, 0:G, :],
                    wt8[:, k0 * O_PER:(k0 + G) * O_PER],
                )
                w_tiles.append((k0, G, wt_t))
                k0 += G

            # warm the PE clock gate with dummy matmuls on a memset tile
            warm = consts.tile([128, WARM_N], dt.bfloat16, name="warm")
            nc.vector.memset(warm[:], 1.0)
            ps_warm = psump.tile([M, WARM_N], dt.float32, name="psw", tag="psw")
            for _ in range(N_WARM):
                nc.tensor.matmul(ps_warm[:], warm[:, 0:M], warm[:],
                                 start=True, stop=True)

            psums = [
                psump.tile([M, w], dt.float32, name=f"ps{i}", tag=f"ps{i}")
                for i, (_, w, _e) in enumerate(CHUNKS)
            ]

            out_rings = [nc.sync, nc.scalar, nc.gpsimd]

            def epilogue(i, o, w):
                # psum rows 0:16 already hold the final scaled+biased
                # output: one PSUM->SBUF copy, then DMA.  Chunk 2 copies
                # on DVE so the tail chains use different engines.
                comb = outp.tile([BATCH, w], dt.float32, name=f"comb{i}")
                if i == 2:
                    nc.vector.tensor_copy(comb[:], psums[i][0:BATCH, :])
                else:
                    nc.scalar.copy(comb[:], psums[i][0:BATCH, :])
                out_rings[i].dma_start(out[:][:, o:o + w], comb[:])

            for k0, G, wt_t in w_tiles:
                # decode (c-128)*s: DVE cols [0,864), ACT cols [864,1376)
                wbA = wbAp.tile([128, GMAX, DVE_W], dt.bfloat16, tag="wA")
                nc.vector.tensor_scalar(
                    wbA[:, 0:G, :], wt_t[:, 0:G, 0:DVE_W],
                    -128.0, sc_sb[:, 0:1], alu.add, alu.mult,
                )
                wbB = wbBp.tile([128, GMAX, ACT_W], dt.bfloat16, tag="wB")
                nc.scalar.activation(
                    wbB[:, 0:G, :], wt_t[:, 0:G, DVE_W:O_PER],
                    mybir.ActivationFunctionType.Identity,
                    bias=sc_sb[:, 1:2], scale=sc_sb[:, 0:1],
                )
                for t in range(G):
                    k = k0 + t
                    last = k == K_TILES - 1

                    def mv_of(i):
                        o, w, eng = CHUNKS[i]
                        if eng == "dve":
                            return wbA[:, t, o:o + w]
                        return wbB[:, t, o - DVE_W:o - DVE_W + w]

                    if k == BIAS_K:
                        # K=2 bias matmuls (bias_hi + bias_lo), off the
                        # critical tail
                        for i, (o, w, _e) in enumerate(CHUNKS):
                            nc.tensor.matmul(
                                psums[i][:, :],
                                x_sb[0:2, BIASBLK * M:(BIASBLK + 1) * M],
                                bias_sb[0:2, o:o + w],
                                start=False,
                                stop=False,
                            )
                    if not last:
                        for i in range(len(CHUNKS)):
                            nc.tensor.matmul(
                                psums[i][:, :],
                                x_sb[:, k * M:(k + 1) * M],
                                mv_of(i),
                                start=(k == 0),
                                stop=False,
                            )
                    else:
                        # close chunk-by-chunk, smallest chunk last so the
                        # final serial epilogue chain is the shortest
                        for i in (0, 2, 1):
                            o, w, _e = CHUNKS[i]
                            nc.tensor.matmul(
                                psums[i][:, :],
                                x_sb[:, k * M:(k + 1) * M],
                                mv_of(i),
                                start=False,
                                stop=True,
                            )
                            epilogue(i, o, w)

    nc.compile()
    return nc


def _get_built():
    global _BUILT
    if _BUILT is None:
        _BUILT = _build()
    return _BUILT


def make_in_maps(x, w_q, scale, bias):
    """Host-side shard + layout prep. Returns per-core input dicts."""
    x = np.asarray(x, dtype=np.float32)
    w_q = np.asarray(w_q, dtype=np.int32)
    scale = np.asarray(scale, dtype=np.float32)
    bias = np.asarray(bias, dtype=np.float32)
    s = float(scale.reshape(-1)[0])

    xT = np.ascontiguousarray(x.T)  # [4096, 16]
    x16 = xT.astype(ml_dtypes.bfloat16)
    # prepack to the SBUF layout [128, K_TILES*M]: partition p holds,
    # for each k-tile t, the stationary block row (t*128 + p)
    xt2 = np.zeros((128, (K_TILES + 1) * M), dtype=ml_dtypes.bfloat16)
    xt2[:, :K_TILES * M] = (
        x16.reshape(K_TILES, 128, M).transpose(1, 0, 2).reshape(128, K_TILES * M)
    )
    # bias one-hot block: partitions 0 and 1, all BATCH stationary
    # columns = 1 (K=2 matmul adds bias_hi + bias_lo)
    xt2[0, K_TILES * M:K_TILES * M + BATCH] = 1.0
    xt2[1, K_TILES * M:K_TILES * M + BATCH] = 1.0

    sc2 = np.zeros((128, 2), dtype=np.float32)
    sc2[:, 0] = s
    sc2[:, 1] = -128.0 * s

    in_maps = []
    for c in range(N_CORES):
        # uint8 codes, transposed to [4096, 1376] then packed so partition
        # p holds, for k-tile t, row (t*128 + p): [128, 32*1376]
        wt_c = w_q[c * O_PER:(c + 1) * O_PER].T.astype(np.uint8)
        wt8_c = np.ascontiguousarray(
            wt_c.reshape(K_TILES, 128, O_PER)
            .transpose(1, 0, 2)
            .reshape(128, K_TILES * O_PER)
        )
        b = bias[c * O_PER:(c + 1) * O_PER]
        bh = b.astype(ml_dtypes.bfloat16)
        bl = (b - bh.astype(np.float32)).astype(ml_dtypes.bfloat16)
        bias_hl_c = np.ascontiguousarray(np.stack([bh, bl], axis=0))
        in_maps.append(
            {"wt8": wt8_c, "xt2": xt2, "bias_hl": bias_hl_c, "sc2": sc2}
        )
    return in_maps


def run(inputs, trace=False):
    """Run on the 8 NeuronCores. Returns (full_output, BassKernelResults)."""
    from concourse.bass_utils import run_bass_kernel_spmd

    in_maps = make_in_maps(**inputs)
    nc = _get_built()
    res = run_bass_kernel_spmd(nc, in_maps, list(range(N_CORES)), trace=trace)
    parts = [np.asarray(res.results[c]["out"]) for c in range(N_CORES)]
    full = np.concatenate(parts, axis=1)[:, :OUT_F].astype(np.float32)
    return full, res


def kernel(**inputs) -> np.ndarray:
    full, _ = run(inputs, trace=False)
    return full
